# revision 4
# baseline (speedup 1.0000x reference)
"""GCNAlign 2-layer GCN forward on 8 trn2 NeuronCores — v2.

The v1 trace showed the kernel is bound by GpSimd SWDGE descriptor
generation for the edge gathers (~8.2ns/row, engine-serial; 392 calls x
8.7us = 3.4ms) with the Vector/Scalar engines additionally burning
~3.5ms building one-hot scatter matrices (mostly stalled behind the
gathers). v2 restructures so everything except the irreducible
descriptor generation is off the critical path:

  - ew == 1/in_degree(tgt) is a per-TARGET constant (host-verified):
    one-hot matrices become exact 0/1 (fp8!) and the edge weight is
    applied once per target tile, fused into the PSUM->SBUF copy.
  - One-hot matrices are precomputed on host (with duplicate-source
    slots merged into integer multiplicities) and streamed from HBM as
    fp8 instead of being built on-device.
  - Gather idx lists are 0-padded at 16-granularity (the sequencer's
    descriptor-ring accounting uses the static index count, so runtime
    negative-index trimming would desync the ring).
  - W matmuls in fp16 (v1 used fp32 = 4 cycles/row on the PE).
  - AllGathers are emitted so their transfers overlap the other
    branch's gather stream.
"""

import os
import heapq
import numpy as np

import concourse.bass as bass
import concourse.bacc as bacc
import concourse.mybir as mybir
from concourse.tile import TileContext
from concourse import bass_utils

F32 = mybir.dt.float32
F16 = mybir.dt.float16
F8 = mybir.dt.float8e4
I16 = mybir.dt.int16

N_NODES = 50000
N_EDGES = 800000
DIM = 200
N_CORES = 8
NPC = N_NODES // N_CORES  # 6250
HALF = N_NODES // 2
DPAD = 256
AT = mybir.ActivationFunctionType
OP = mybir.AluOpType


def tile_sizes_for(npc):
    sizes = [128] * (npc // 128)
    if npc % 128:
        sizes.append(npc % 128)
    return sizes


# ---------------------------------------------------------------------------
# Host-side planning
# ---------------------------------------------------------------------------

def plan_branch(edges, ew, n_nodes, n_cores, sizes):
    """Balanced node->tile assignment (greedy by in-degree, as v1)."""
    src = np.asarray(edges[0], dtype=np.int64)
    tgt = np.asarray(edges[1], dtype=np.int64)
    ew = np.asarray(ew, dtype=np.float32).reshape(-1)
    T = len(sizes)
    n_tiles = n_cores * T
    caps = np.tile(np.asarray(sizes, dtype=np.int64), n_cores)
    assert caps.sum() == n_nodes

    deg = np.bincount(tgt, minlength=n_nodes)
    order = np.argsort(-deg, kind="stable")
    heap = [(0, t) for t in range(n_tiles)]
    heapq.heapify(heap)
    remaining = caps.copy()
    tile_of_node = np.empty(n_nodes, dtype=np.int32)
    tile_members = [[] for _ in range(n_tiles)]
    for node in order:
        while True:
            s, t = heapq.heappop(heap)
            if remaining[t] > 0:
                break
        tile_of_node[node] = t
        tile_members[t].append(node)
        remaining[t] -= 1
        if remaining[t] > 0:
            heapq.heappush(heap, (s + int(deg[node]), t))

    def _layout(members):
        perm = np.concatenate([np.asarray(m, dtype=np.int64) for m in members])
        inv_perm = np.empty(n_nodes, dtype=np.int64)
        inv_perm[perm] = np.arange(n_nodes)
        tof = np.empty(n_nodes, dtype=np.int32)
        for g, m in enumerate(members):
            tof[np.asarray(m, dtype=np.int64)] = g
        tile_starts_nodes = np.concatenate([[0], np.cumsum(caps)])
        loc = (inv_perm - tile_starts_nodes[tof]).astype(np.int32)
        e_tile = tof[tgt]
        e_order = np.argsort(e_tile, kind="stable")
        bounds = np.searchsorted(e_tile[e_order], np.arange(n_tiles + 1))
        return perm, inv_perm, loc, e_order, bounds

    # Pass 1: provisional layout to measure per-(core,tile) half-A distinct
    # source counts. Reordering slots WITHIN a core does not move any node
    # across the half boundary (each core's rows sit wholly in one table
    # half), so the counts are invariant under the within-core sort below.
    perm, inv_perm, loc_of_node, e_order, bounds = _layout(tile_members)
    half = n_nodes // 2
    src_new = inv_perm[src]
    T = len(sizes)
    a_cnt = np.zeros((n_cores, T), np.int64)
    for g in range(n_tiles):
        s = src_new[e_order[bounds[g]:bounds[g + 1]]]
        a_cnt[g // T, g % T] = len(np.unique(s[s < half]))
    # Per-core sort of the full-size slots by half-A count: slot t then holds
    # similarly-sized tiles on every core, so the cross-core max that sizes
    # the static gather calls hugs the mean instead of the +2-sigma tail.
    members2 = []
    for c in range(n_cores):
        order = np.argsort(a_cnt[c, :T - 1], kind="stable")
        members2.extend(tile_members[c * T + t] for t in order)
        members2.append(tile_members[c * T + T - 1])
    tile_members = members2
    perm, inv_perm, loc_of_node, e_order, bounds = _layout(tile_members)

    # per-target weight (ew must be a pure function of the target node)
    w_node = np.zeros(n_nodes, np.float32)
    w_node[tgt] = ew
    resid = np.abs(ew - w_node[tgt])
    scale_ok = bool(np.max(resid) <= 1e-6 * max(1.0, np.max(np.abs(ew))))

    return {
        "perm": perm, "inv_perm": inv_perm, "loc_of_node": loc_of_node,
        "e_order": e_order, "bounds": bounds, "src": src, "tgt": tgt,
        "ew": ew, "w_node": w_node, "scale_ok": scale_ok,
    }


def plan_v2(plans, n_cores, sizes):
    """Per (core, tile, half): dedup'd gather slots + multi-hot matrices.

    Layout per branch:
      idx16 [cores, 128, C16]  int16 row-within-half, -1 trailing pad
      oh    [cores, 128, B]    fp8 multiplicity one-hot blob, tile-major
                               (half-a chunks then half-b chunks)
      wvec  [cores, 128, T]    f32 per-target weight for tile column t
    Shapes (ni/cf per tile/half) are max'd across cores and branches so a
    single SPMD program fits all cores.
    """
    T = len(sizes)
    out = []
    for plan in plans:
        e_order, bounds = plan["e_order"], plan["bounds"]
        src_new = plan["inv_perm"][plan["src"]]
        loc_tgt = plan["loc_of_node"][plan["tgt"]]
        data = {}
        nslot = np.zeros((T, 2), np.int64)
        for g in range(n_cores * T):
            c, t = g // T, g % T
            sl = e_order[bounds[g]:bounds[g + 1]]
            s = src_new[sl]
            lt = loc_tgt[sl]
            for h in range(2):
                m = (s >= HALF) if h else (s < HALF)
                sh, lh = s[m] - h * HALF, lt[m]
                # dedup: slots = distinct sources; one-hot carries counts
                # per (source, target) pair.
                if len(sh):
                    uniq, slot_of = np.unique(sh, return_inverse=True)
                else:
                    uniq = np.zeros(0, np.int64)
                    slot_of = np.zeros(0, np.int64)
                data[(c, t, h)] = (uniq, slot_of, lh)
                nslot[t, h] = max(nslot[t, h], len(uniq))
        out.append({"data": data, "nslot": nslot})

    # num_idxs is 16-granular (0-padded: the ring-space accounting on the
    # sequencer uses the static count, so runtime -1 trimming would desync
    # the descriptor ring); chunk count rounds up to 128 for the PE.
    # Shapes are per-branch (max over cores only).
    ni_br, cf_br, C16_br, B_br = [], [], [], []
    for br in range(2):
        nslot = out[br]["nslot"]
        ni = np.maximum(((nslot + 15) // 16) * 16, 16)
        cf = (ni + 127) // 128
        ni_br.append(ni)
        cf_br.append(cf)
        C16_br.append(int(ni.sum()) // 16)
        B_br.append(int(cf.sum()) * 128)

    res = []
    for br, plan in enumerate(plans):
        data = out[br]["data"]
        ni_th, cf_th = ni_br[br], cf_br[br]
        C16, B = C16_br[br], B_br[br]
        idx16 = np.zeros((n_cores, 128, C16), np.int16)
        oh = np.zeros((n_cores, 128, B), np.float32)
        wvec = np.zeros((n_cores, 128, T), np.float32)
        cnt = np.zeros((n_cores, 1, 2 * T), np.int32)
        w_node = plan["w_node"]
        perm = plan["perm"]
        for c in range(n_cores):
            col16 = 0
            ohcol = 0
            for t in range(T):
                for h in range(2):
                    uniq, slot_of, lh = data[(c, t, h)]
                    ni = int(ni_th[t, h])
                    n = len(uniq)
                    assert n <= ni
                    cnt[c, 0, 2 * t + h] = n
                    idx = np.zeros(ni, np.int16)
                    idx[:n] = uniq.astype(np.int16)
                    cols = ni // 16
                    seg = idx.reshape(cols, 16).T  # [16, cols]
                    idx16[c, :, col16:col16 + cols] = np.tile(seg, (8, 1))
                    col16 += cols
                    ncf = int(cf_th[t, h])
                    ohm = np.zeros((128, ncf * 128), np.float32)
                    np.add.at(ohm, (slot_of % 128,
                                    (slot_of // 128) * 128 + lh), 1.0)
                    oh[c, :, ohcol:ohcol + ncf * 128] = ohm
                    ohcol += ncf * 128
                off = int(np.sum(sizes[:t]))
                sz = sizes[t]
                rows = perm[c * NPC + off:c * NPC + off + sz]
                wvec[c, :sz, t] = w_node[rows]
            assert col16 == C16 and ohcol == B
        res.append({"idx16": idx16, "oh": oh, "wvec": wvec, "cnt": cnt})

    meta = {"cf_th": cf_br, "ni_th": ni_br, "C16": C16_br, "B": B_br}
    return res, meta


# ---------------------------------------------------------------------------
# Bass kernel builder
# ---------------------------------------------------------------------------

def build_gcn(n_cores, sizes, meta, tab_dt=F16, tail16=False, oh_dt=F8):
    T = len(sizes)
    cf_br, ni_br = meta["cf_th"], meta["ni_th"]
    C16_br, B_br = meta["C16"], meta["B"]
    TT = F16 if tail16 else F32  # transpose/v-matmul dtype

    nc = bacc.Bacc("TRN2", target_bir_lowering=False, debug=False,
                   num_devices=n_cores)
    rg = [list(range(n_cores))]

    emb_in, idx_in, oh_in, w_in_br, out_ext = {}, {}, {}, {}, {}
    for br in range(2):
        emb_in[br] = nc.dram_tensor(f"emb{br}", [NPC, DIM], F32,
                                    kind="ExternalInput")
        idx_in[br] = nc.dram_tensor(f"idx{br}", [128, C16_br[br]], I16,
                                    kind="ExternalInput")
        oh_in[br] = nc.dram_tensor(f"oh{br}", [128, B_br[br]], oh_dt,
                                   kind="ExternalInput")
        w_in_br[br] = nc.dram_tensor(f"wv{br}", [128, T], F32,
                                     kind="ExternalInput")
        cnt_in = nc.dram_tensor(f"cnt{br}", [1, 2 * T], mybir.dt.int32,
                                kind="ExternalInput")
        idx_in[f"cnt{br}"] = cnt_in
        out_ext[br] = nc.dram_tensor(f"out{br}", [NPC, DIM], F32,
                                     kind="ExternalOutput")
    w_in = nc.dram_tensor("conv_w", [DIM, DIM], F16 if tail16 else F32,
                          kind="ExternalInput")
    b_in = nc.dram_tensor("conv_b", [128, DIM], F32, kind="ExternalInput")

    row_slices = []
    off = 0
    for sz in sizes:
        row_slices.append((off, sz))
        off += sz
    assert off == NPC

    with TileContext(nc) as tc:
        with (
            tc.tile_pool(name="const", bufs=1) as cpool,
            tc.tile_pool(name="dram", bufs=1, space="DRAM") as dpool,
            tc.tile_pool(name="work", bufs=4) as work,
            tc.tile_pool(name="gbuf", bufs=4) as gpool,
            tc.tile_pool(name="ohp", bufs=6) as ohpool,
            tc.tile_pool(name="psu", bufs=2, space="PSUM") as psU,
            tc.tile_pool(name="pst", bufs=2, space="PSUM") as psT,
            tc.tile_pool(name="psv", bufs=2, space="PSUM") as psV,
            tc.tile_pool(name="outs", bufs=4) as outp,
        ):
            # ---- constants ----
            w_a = cpool.tile([128, DIM], TT)
            nc.sync.dma_start(w_a[:], w_in[0:128, :])
            w_b = cpool.tile([DIM - 128, DIM], TT)
            nc.sync.dma_start(w_b[:], w_in[128:DIM, :])
            bb = cpool.tile([128, DIM], F32)
            nc.sync.dma_start(bb[:], b_in[:, :])
            iota_i = cpool.tile([128, 128], mybir.dt.int32)
            nc.gpsimd.iota(iota_i[:], pattern=[[1, 128]], channel_multiplier=0)
            iota_f = cpool.tile([128, 128], F32)
            nc.vector.tensor_copy(iota_f[:], iota_i[:])
            pidx_i = cpool.tile([128, 1], mybir.dt.int32)
            nc.gpsimd.iota(pidx_i[:], pattern=[[0, 1]], channel_multiplier=1)
            pidx_f = cpool.tile([128, 1], F32)
            nc.vector.tensor_copy(pidx_f[:], pidx_i[:])
            ident = cpool.tile([128, 128], TT)
            nc.vector.tensor_scalar(
                out=ident[:], in0=iota_f[:], scalar1=pidx_f[:, :1],
                scalar2=None, op0=OP.is_equal)

            idx_sb, wv_sb, cnt_sb = {}, {}, {}
            for br in range(2):
                idx_sb[br] = cpool.tile([128, C16_br[br]], I16,
                                        name=f"idxsb{br}")
                nc.scalar.dma_start(idx_sb[br][:], idx_in[br][:, :])
                wv_sb[br] = cpool.tile([128, T], F32, name=f"wvsb{br}")
                nc.scalar.dma_start(wv_sb[br][:], w_in_br[br][:, :])
                cnt_sb[br] = cpool.tile([1, 2 * T], mybir.dt.int32,
                                        name=f"cntsb{br}")
                nc.scalar.dma_start(cnt_sb[br][:], idx_in[f"cnt{br}"][:, :])

            cnt_reg_a = nc.alloc_register(mybir.EngineType.Pool, "gcnt_a")
            cnt_reg_b = nc.alloc_register(mybir.EngineType.Pool, "gcnt_b")

            tcf_max = int(max((cf_br[b][:, 0] + cf_br[b][:, 1]).max()
                              for b in range(2)))

            # ---- DRAM tables ----
            x0_shard, x0_tab, x1_shard, x1_tab = {}, {}, {}, {}
            for br in range(2):
                x0_shard[br] = dpool.tile([NPC, DPAD], tab_dt, name=f"x0s{br}")
                x0_tab[br] = dpool.tile([N_NODES, DPAD], tab_dt,
                                        addr_space="Shared", name=f"x0t{br}")
                x1_shard[br] = dpool.tile([NPC, DPAD], tab_dt, name=f"x1s{br}")
                x1_tab[br] = dpool.tile([N_NODES, DPAD], tab_dt,
                                        addr_space="Shared", name=f"x1t{br}")

            def normalize(br):
                for t in range(T):
                    off, sz = row_slices[t]
                    e_t = work.tile([128, DIM], F32, tag="nrm_in")
                    nc.sync.dma_start(e_t[:sz], emb_in[br][off:off + sz, :])
                    sq = work.tile([128, DIM], F32, tag="nrm_sq")
                    ssq = work.tile([128, 1], F32, tag="nrm_ssq")
                    nc.scalar.activation(sq[:sz], e_t[:sz], AT.Square,
                                         accum_out=ssq[:sz])
                    nrm = work.tile([128, 1], F32, tag="nrm_n")
                    nc.scalar.activation(nrm[:sz], ssq[:sz], AT.Sqrt)
                    nc.vector.tensor_scalar_max(nrm[:sz], nrm[:sz], 1e-12)
                    inv = work.tile([128, 1], F32, tag="nrm_i")
                    nc.vector.reciprocal(inv[:sz], nrm[:sz])
                    xo = outp.tile([128, DPAD], tab_dt, tag="nrm_out")
                    nc.vector.tensor_scalar_mul(xo[:sz, 0:DIM], e_t[:sz],
                                                inv[:sz, :1])
                    nc.gpsimd.dma_start(x0_shard[br][off:off + sz, :],
                                        xo[:sz, :])

            def allgather(shard, tab):
                nc.gpsimd.collective_compute(
                    "AllGather", OP.bypass, replica_groups=rg,
                    ins=[shard[:]], outs=[tab[:]])

            def layer(br, tab, dst, last):
                cf_th, ni_th = cf_br[br], ni_br[br]
                colo = 0
                ohcol = 0
                for t in range(T):
                    off, sz = row_slices[t]
                    cfa, cfb = int(cf_th[t, 0]), int(cf_th[t, 1])
                    nia, nib = int(ni_th[t, 0]), int(ni_th[t, 1])
                    tcf = cfa + cfb
                    g = gpool.tile([128, tcf_max, DPAD], tab_dt, tag="G")
                    nc.gpsimd.dma_gather(
                        g[:, 0:cfa, :], tab[0:HALF, :],
                        idx_sb[br][:, colo:colo + nia // 16],
                        nia, nia, DPAD, single_packet=False)
                    nc.gpsimd.dma_gather(
                        g[:, cfa:tcf, :], tab[HALF:N_NODES, :],
                        idx_sb[br][:, colo + nia // 16:(colo + (nia + nib)
                                                        // 16)],
                        nib, nib, DPAD, single_packet=False)
                    colo += (nia + nib) // 16
                    oh = ohpool.tile([128, tcf_max * 128], oh_dt, tag="OH")
                    nc.scalar.dma_start(
                        oh[:, 0:tcf * 128],
                        oh_in[br][:, ohcol:ohcol + tcf * 128])
                    ohcol += tcf * 128
                    u = psU.tile([128, DIM], F32, tag="u")
                    for c in range(tcf):
                        nc.tensor.matmul(
                            u[:], lhsT=oh[:, c * 128:(c + 1) * 128],
                            rhs=g[:, c, 0:DIM],
                            start=(c == 0), stop=(c == tcf - 1))
                    us = work.tile([128, DIM], TT, tag="us")
                    nc.vector.tensor_scalar_mul(us[:], u[:],
                                                wv_sb[br][:, t:t + 1])
                    ut = psT.tile([128, 256], TT, tag="ut")
                    nc.tensor.transpose(ut[:, 0:128], us[:, 0:128], ident[:])
                    nc.tensor.transpose(ut[0:DIM - 128, 128:256],
                                        us[:, 128:DIM], ident[:])
                    uts = work.tile([128, 256], TT, tag="uts")
                    nc.vector.tensor_copy(uts[:, 0:128], ut[:, 0:128])
                    nc.vector.tensor_copy(uts[0:DIM - 128, 128:256],
                                          ut[0:DIM - 128, 128:256])
                    v = psV.tile([128, DIM], F32, tag="v")
                    nc.tensor.matmul(v[:], lhsT=uts[:, 0:128], rhs=w_a[:],
                                     start=True, stop=False)
                    nc.tensor.matmul(v[:], lhsT=uts[0:DIM - 128, 128:256],
                                     rhs=w_b[:], start=False, stop=True)
                    xa = work.tile([128, DIM], F32, tag="xa")
                    nc.vector.tensor_tensor(xa[:], v[:], bb[:], op=OP.add)
                    if last:
                        xo = outp.tile([128, DIM], F32, tag="xo2")
                        nc.scalar.activation(xo[:], xa[:], AT.Relu)
                        nc.sync.dma_start(dst[off:off + sz, :], xo[:sz])
                    else:
                        xo = outp.tile([128, DPAD], tab_dt, tag="xo1")
                        nc.scalar.activation(xo[:, 0:DIM], xa[:], AT.Relu)
                        nc.sync.dma_start(dst[off:off + sz, :], xo[:sz, :])

            for br in range(2):
                normalize(br)
                allgather(x0_shard[br], x0_tab[br])
            # prime gather buffers: slots the (0-padded) gathers never write
            # must hold finite values — 0 * NaN would poison the matmul.
            # Runs on DVE during the x0 AllGather window.
            for _ in range(4):
                gz = gpool.tile([128, tcf_max, DPAD], tab_dt, tag="G")
                nc.vector.memset(gz[:], 0.0)
            layer(0, x0_tab[0], x1_shard[0], last=False)
            allgather(x1_shard[0], x1_tab[0])
            layer(1, x0_tab[1], x1_shard[1], last=False)
            allgather(x1_shard[1], x1_tab[1])
            layer(0, x1_tab[0], out_ext[0], last=True)
            layer(1, x1_tab[1], out_ext[1], last=True)

    nc.compile()
    return nc


# ---------------------------------------------------------------------------
# Entry point
# ---------------------------------------------------------------------------

def _run(match_emb, ref_emb, conv_w, conv_b, match_edges, ref_edges,
         match_ew, ref_ew, trace=False, tail16=False, oh8=True):
    sizes = tile_sizes_for(NPC)
    plans = [plan_branch(e, w, N_NODES, N_CORES, sizes)
             for e, w in ((match_edges, match_ew), (ref_edges, ref_ew))]
    assert plans[0]["scale_ok"] and plans[1]["scale_ok"], \
        "ew is not a per-target constant; v2 fast path invalid"
    res_pl, meta = plan_v2(plans, N_CORES, sizes)

    oh_dt = F8 if oh8 else F16
    nc = build_gcn(N_CORES, sizes, meta, tail16=tail16, oh_dt=oh_dt)

    embs = [np.asarray(match_emb, np.float32), np.asarray(ref_emb, np.float32)]
    emb_perm = [embs[b][plans[b]["perm"]] for b in range(2)]
    b_bcast = np.ascontiguousarray(
        np.broadcast_to(np.asarray(conv_b, np.float32)[None, :], (128, DIM)))
    w_np = np.ascontiguousarray(np.asarray(conv_w, np.float32))
    if tail16:
        w_np = w_np.astype(np.float16)

    in_maps = []
    for c in range(N_CORES):
        m = {"conv_w": w_np, "conv_b": b_bcast}
        for br in range(2):
            m[f"emb{br}"] = np.ascontiguousarray(
                emb_perm[br][c * NPC:(c + 1) * NPC])
            m[f"idx{br}"] = np.ascontiguousarray(res_pl[br]["idx16"][c])
            m[f"oh{br}"] = np.ascontiguousarray(
                res_pl[br]["oh"][c].astype(mybir.dt.np(oh_dt)))
            m[f"cnt{br}"] = np.ascontiguousarray(res_pl[br]["cnt"][c])
            m[f"wv{br}"] = np.ascontiguousarray(res_pl[br]["wvec"][c])
        in_maps.append(m)

    res = bass_utils.run_bass_kernel_spmd(
        nc, in_maps, core_ids=list(range(N_CORES)), trace=trace)

    outs = []
    for br in range(2):
        full = np.empty((N_NODES, DIM), dtype=np.float32)
        perm = plans[br]["perm"]
        for c in range(N_CORES):
            full[perm[c * NPC:(c + 1) * NPC]] = res.results[c][f"out{br}"]
        outs.append(full)
    return (outs[0], outs[1]), res


def kernel(match_emb, ref_emb, conv_w, conv_b, match_edges, ref_edges,
           match_ew, ref_ew):
    trace = bool(int(os.environ.get("KERNEL_TRACE", "0")))
    tail16 = bool(int(os.environ.get("GCN_TAIL16", "0")))
    oh8 = bool(int(os.environ.get("GCN_OH8", "1")))
    (out_m, out_r), _ = _run(match_emb, ref_emb, conv_w, conv_b,
                             match_edges, ref_edges, match_ew, ref_ew,
                             trace=trace, tail16=tail16, oh8=oh8)
    return out_m, out_r
